# revision 8
# baseline (speedup 1.0000x reference)
"""CRF decoder loss kernel for Trainium2 (8 NeuronCores, data-parallel over batch).

Algorithm (mathematically identical to the reference):
  The reference computes mean_b(Zp - score) where Zp is the CRF partition
  function of log_softmax(enc@W+b) and score is the gold-path score. Writing
  logits = R - logZ (R the raw projection scores, logZ the log-softmax
  normalizer), the normalizer cancels between Zp and score, so no softmax is
  ever needed. With a constant shift kappa for range control, the forward
  recursion runs in LINEAR space:

      P_0 = exp(start) * G_0,     P_t = (P_{t-1} @ exp(T)) * G_t,
      G_t = exp(R_t - kappa)                                  (all [B, V])

  loss_b = log(sum_j P_{len_b-1}[b,j] * exp(end_j))           <- S, device
           - sum_{t<len_b} (R[t,b,tgt_{t,b}] - kappa)         <- host (tiny)
           - (start[tgt_0] + sum T[tgt,tgt'] + end[tgt_last]) <- host (tiny)

  Validated vs the reference: f64 exact (1e-16); with bf16 device dtypes the
  loss rel-err is ~1e-6.

Device work per core (batch shard of 32, v-major layouts):
  - projection: R^T = W^T @ encT into PSUM (bf16 matmuls, fp32 accum),
    ACT evicts G^T = exp(R^T + (b - kappa)) as bf16.
  - scan: state P^T [v, 32] bf16 in a 32-slot ring; per step 4 matmuls with
    the four 128x128 blocks of exp(T) stationary + one DVE multiply by G_t^T.
    Two independent 16-batch groups interleave to hide cross-engine latency.
  - S extraction: every 16 steps a batched matmul with exp(end) over the ring
    yields S_t[b] for all (t, b); host picks t = len_b - 1.
"""

import numpy as np
import ml_dtypes

import concourse.bacc as bacc
import concourse.tile as tile
from concourse import mybir
from concourse.bass_utils import run_bass_kernel_spmd

bf16 = ml_dtypes.bfloat16
f32 = mybir.dt.float32
bf16_t = mybir.dt.bfloat16

S, B, H, V = 512, 256, 512, 256
NCORES = 8
BC = B // NCORES            # 32 batch per core
ROWS = S * BC               # 16384 rows (t-major, b-minor)
KAPPA = 6.05
CHUNK = 512                 # projection chunk (rows)
NCHUNK = ROWS // CHUNK      # 32
NG = 2                      # scan batch groups per core
GB = BC // NG               # 16
SBLK = 16                   # scan steps per S-extraction block
RING = 32                   # state ring slots

_nc_cache = None


def _build():
    nc = bacc.Bacc("TRN2", debug=False)

    encT = nc.dram_tensor("encT", [128, NCHUNK, 4, CHUNK], bf16_t, kind="ExternalInput")
    wblk = nc.dram_tensor("wblk", [128, 8, 128], bf16_t, kind="ExternalInput")
    expTblk = nc.dram_tensor("expTblk", [128, 4, 128], bf16_t, kind="ExternalInput")
    biasT = nc.dram_tensor("biasT", [128, 2], f32, kind="ExternalInput")
    expStartT = nc.dram_tensor("expStartT", [128, 2], f32, kind="ExternalInput")
    expEndT = nc.dram_tensor("expEndT", [128, 2], bf16_t, kind="ExternalInput")

    s_out = nc.dram_tensor("s_out", [1, ROWS], f32, kind="ExternalOutput")

    with tile.TileContext(nc) as tc:
        with (
            tc.tile_pool(name="consts", bufs=1) as consts,
            tc.tile_pool(name="encp", bufs=3) as encp,
            tc.tile_pool(name="gpool", bufs=1) as gpool,
            tc.tile_pool(name="proj_ps", bufs=3, space="PSUM") as proj_ps,
            tc.tile_pool(name="scan_ps", bufs=2, space="PSUM") as scan_ps,
            tc.tile_pool(name="s_ps", bufs=1, space="PSUM") as s_ps,
        ):
            w_sb = consts.tile([128, 8, 128], bf16_t)
            expT_sb = consts.tile([128, 4, 128], bf16_t)
            bias_sb = consts.tile([128, 2], f32)
            expStart_sb = consts.tile([128, 2], f32)
            expEnd_sb = consts.tile([128, 2], bf16_t)
            s_sb = consts.tile([1, ROWS], f32)
            rings = [consts.tile([128, RING, 2, GB], bf16_t, name=f"ring{g}",
                                 tag=f"ring{g}")
                     for g in range(NG)]

            nc.sync.dma_start(out=w_sb[:], in_=wblk[:])
            nc.sync.dma_start(out=expT_sb[:], in_=expTblk[:])
            nc.sync.dma_start(out=bias_sb[:], in_=biasT[:])
            nc.sync.dma_start(out=expStart_sb[:], in_=expStartT[:])
            nc.sync.dma_start(out=expEnd_sb[:], in_=expEndT[:])

            # ---------------- projection ----------------
            gtiles = []
            for c in range(NCHUNK):
                et = encp.tile([128, 4, CHUNK], bf16_t, name="et", tag="enc")
                nc.sync.dma_start(out=et[:], in_=encT[:, c, :, :])
                g = gpool.tile([128, 2, CHUNK], bf16_t, name=f"g{c}", tag=f"g{c}")
                gtiles.append(g)
                for vh in range(2):
                    ps = proj_ps.tile([128, CHUNK], f32, name="pps", tag="pps")
                    for ht in range(4):
                        nc.tensor.matmul(
                            ps[:],
                            lhsT=w_sb[:, ht * 2 + vh, :],
                            rhs=et[:, ht, :],
                            start=(ht == 0),
                            stop=(ht == 3),
                        )
                    nc.scalar.activation(
                        g[:, vh, :], ps[:],
                        mybir.ActivationFunctionType.Exp,
                        bias=bias_sb[:, vh:vh + 1], scale=1.0,
                    )

            # ---------------- scan ----------------
            for gi in range(NG):
                for ih in range(2):
                    nc.vector.tensor_scalar_mul(
                        rings[gi][:, 0, ih, :],
                        in0=gtiles[0][:, ih, gi * GB:(gi + 1) * GB],
                        scalar1=expStart_sb[:, ih:ih + 1],
                    )

            def emit_sblock(k):
                # S_t for steps t in [k*SBLK, (k+1)*SBLK) from ring slots
                sp = s_ps.tile([1, NG, SBLK * GB], f32, name="sps", tag="sps")
                s0 = (k * SBLK) % RING
                for gi in range(NG):
                    for ih in range(2):
                        nc.tensor.matmul(
                            sp[:, gi, :],
                            lhsT=expEnd_sb[:, ih:ih + 1],
                            rhs=rings[gi][:, s0:s0 + SBLK, ih, :],
                            start=(ih == 0),
                            stop=(ih == 1),
                        )
                nc.scalar.copy(
                    s_sb[0:1, k * (NG * SBLK * GB):(k + 1) * (NG * SBLK * GB)],
                    sp[:].rearrange("one a b -> one (a b)"),
                )

            for t in range(1, S):
                gt = gtiles[t // SBLK]
                off = (t % SBLK) * BC
                for gi in range(NG):
                    ps = scan_ps.tile([128, 2, GB], f32, name=f"scps{gi}",
                                      tag=f"scps{gi}")
                    for jh in range(2):
                        for ih in range(2):
                            nc.tensor.matmul(
                                ps[:, jh, :],
                                lhsT=expT_sb[:, ih * 2 + jh, :],
                                rhs=rings[gi][:, (t - 1) % RING, ih, :],
                                start=(ih == 0),
                                stop=(ih == 1),
                            )
                    nc.vector.tensor_tensor(
                        out=rings[gi][:, t % RING, :, :],
                        in0=ps[:],
                        in1=gt[:, :, off + gi * GB: off + (gi + 1) * GB],
                        op=mybir.AluOpType.mult,
                    )
                if t % SBLK == SBLK - 1:
                    emit_sblock(t // SBLK)

            nc.sync.dma_start(out=s_out[:], in_=s_sb[:])

    nc.compile()
    return nc


def _host_consts(d):
    W_ = np.asarray(d["W"], dtype=np.float32)
    b_ = np.asarray(d["b"], dtype=np.float64)
    T_ = np.asarray(d["transition"], dtype=np.float64)
    start_ = np.asarray(d["start_transition"], dtype=np.float64)
    end_ = np.asarray(d["end_transition"], dtype=np.float64)
    Wb = np.ascontiguousarray(
        W_.reshape(4, 128, 2, 128).transpose(1, 0, 2, 3).reshape(128, 8, 128)
    ).astype(bf16)
    expTb = np.ascontiguousarray(
        np.exp(T_).reshape(2, 128, 2, 128).transpose(1, 0, 2, 3).reshape(128, 4, 128)
    ).astype(bf16)
    biasT = np.ascontiguousarray(
        (b_ - KAPPA).reshape(2, 128).T).astype(np.float32)
    expStartT = np.ascontiguousarray(
        np.exp(start_).reshape(2, 128).T).astype(np.float32)
    expEndT = np.ascontiguousarray(
        np.exp(end_).reshape(2, 128).T).astype(bf16)
    return Wb, expTb, biasT, expStartT, expEndT


def _prep_core_inputs(core, enc_bf, Wb, expTb, biasT, expStartT, expEndT):
    # encT layout [h%128, chunk, h//128, row-in-chunk]; rows are t*BC + b
    b0 = core * BC
    e = enc_bf[:, b0:b0 + BC, :].transpose(2, 0, 1).reshape(4, 128, NCHUNK, CHUNK)
    e = np.ascontiguousarray(e.transpose(1, 2, 0, 3))
    return {
        "encT": e, "wblk": Wb, "expTblk": expTb, "biasT": biasT,
        "expStartT": expStartT, "expEndT": expEndT,
    }


def kernel(enc_outs, W, b, transition, start_transition, end_transition,
           targets, lengths):
    global _nc_cache
    if _nc_cache is None:
        _nc_cache = _build()
    nc = _nc_cache

    enc = np.asarray(enc_outs, dtype=np.float32)
    W_ = np.asarray(W, dtype=np.float32)
    b_ = np.asarray(b, dtype=np.float64)
    T_ = np.asarray(transition, dtype=np.float64)
    start_ = np.asarray(start_transition, dtype=np.float64)
    end_ = np.asarray(end_transition, dtype=np.float64)
    tgt = np.asarray(targets).astype(np.int64)
    lens = np.asarray(lengths).astype(np.int64)

    Wb, expTb, biasT, expStartT, expEndT = _host_consts({
        "W": W, "b": b, "transition": transition,
        "start_transition": start_transition, "end_transition": end_transition,
    })
    enc_bf = enc.astype(bf16)
    in_maps = [
        _prep_core_inputs(c, enc_bf, Wb, expTb, biasT, expStartT, expEndT)
        for c in range(NCORES)
    ]
    res = run_bass_kernel_spmd(nc, in_maps, list(range(NCORES))).results

    # ---------------- host epilogue (small inputs only) ----------------
    tmask = (np.arange(S)[:, None] < lens[None, :])
    trans_sum = (T_[tgt[:-1], tgt[1:]] * tmask[1:]).sum(axis=0)
    last_tgt = tgt[lens - 1, np.arange(B)]
    hostscore = start_[tgt[0]] + trans_sum + end_[last_tgt]

    # gold-path raw emission scores: R[t, b, tgt] = enc[t, b] . W[:, tgt] + b
    # (16K dot products per core; 0.1% of the device FLOPs)
    Wg = W_.T[tgt.reshape(-1)]                        # (S*B, H)
    emis_all = (np.einsum("rh,rh->r", enc.reshape(S * B, H), Wg,
                          optimize=True).reshape(S, B)
                + b_[tgt])
    emis = ((emis_all - KAPPA) * tmask).sum(axis=0)

    loss_b = np.zeros(B, dtype=np.float64)
    for c in range(NCORES):
        b0 = c * BC
        s_flat = np.asarray(res[c]["s_out"], dtype=np.float64).reshape(ROWS)
        # S col layout: block k = t//SBLK, then g, then t%SBLK, then b%GB
        s_dec = s_flat.reshape(S // SBLK, NG, SBLK, GB)
        bl = lens[b0:b0 + BC] - 1
        blocal = np.arange(BC)
        s_end = s_dec[bl // SBLK, blocal // GB, bl % SBLK, blocal % GB]
        loss_b[b0:b0 + BC] = np.log(s_end) - emis[b0:b0 + BC] \
            - hostscore[b0:b0 + BC]

    return np.float32(loss_b.mean())


# revision 9
# speedup vs baseline: 1.1218x; 1.1218x over previous
"""CRF decoder loss kernel for Trainium2 (8 NeuronCores, data-parallel over batch).

Algorithm (mathematically identical to the reference):
  The reference computes mean_b(Zp - score) where Zp is the CRF partition
  function of log_softmax(enc@W+b) and score is the gold-path score. Writing
  logits = R - logZ (R the raw projection scores, logZ the log-softmax
  normalizer), the normalizer cancels between Zp and score, so no softmax is
  ever needed. With a constant shift kappa for range control, the forward
  recursion runs in LINEAR space:

      P_0 = exp(start) * G_0,     P_t = (P_{t-1} @ exp(T)) * G_t,
      G_t = exp(R_t - kappa)                                  (all [B, V])

  loss_b = log(sum_j P_{len_b-1}[b,j] * exp(end_j))           <- S, device
           - sum_{t<len_b} (R[t,b,tgt_{t,b}] - kappa)         <- host (tiny)
           - (start[tgt_0] + sum T[tgt,tgt'] + end[tgt_last]) <- host (tiny)

  Validated vs the reference: f64 exact (1e-16); with bf16 device dtypes the
  loss rel-err is ~1e-6.

Device work per core (batch shard of 32, v-major layouts):
  - projection: R^T = W^T @ encT into PSUM (bf16 matmuls, fp32 accum),
    ACT evicts G^T = exp(R^T + (b - kappa)) as bf16.
  - scan: state P^T [v, 32] bf16 in a 32-slot ring; per step 4 matmuls with
    the four 128x128 blocks of exp(T) stationary + one DVE multiply by G_t^T.
    Two independent 16-batch groups interleave to hide cross-engine latency.
  - S extraction: every 16 steps a batched matmul with exp(end) over the ring
    yields S_t[b] for all (t, b); host picks t = len_b - 1.
"""

import numpy as np
import ml_dtypes

import concourse.bacc as bacc
import concourse.tile as tile
from concourse import mybir
from concourse.bass_utils import run_bass_kernel_spmd

bf16 = ml_dtypes.bfloat16
f32 = mybir.dt.float32
bf16_t = mybir.dt.bfloat16

S, B, H, V = 512, 256, 512, 256
NCORES = 8
BC = B // NCORES            # 32 batch per core
ROWS = S * BC               # 16384 rows (t-major, b-minor)
KAPPA = 6.05
CHUNK = 512                 # projection chunk (rows)
NCHUNK = ROWS // CHUNK      # 32
NG = 2                      # scan batch groups per core
GB = BC // NG               # 16
SBLK = 16                   # scan steps per S-extraction block
RING = 32                   # state ring slots

_nc_cache = None


def _build():
    nc = bacc.Bacc("TRN2", debug=False)

    encT = nc.dram_tensor("encT", [128, NCHUNK, 4, CHUNK], bf16_t, kind="ExternalInput")
    wblk = nc.dram_tensor("wblk", [128, 8, 128], bf16_t, kind="ExternalInput")
    expTblk = nc.dram_tensor("expTblk", [128, 4, 128], bf16_t, kind="ExternalInput")
    biasT = nc.dram_tensor("biasT", [128, 2], f32, kind="ExternalInput")
    expStartT = nc.dram_tensor("expStartT", [128, 2], f32, kind="ExternalInput")
    expEndT = nc.dram_tensor("expEndT", [128, 2], bf16_t, kind="ExternalInput")

    s_out = nc.dram_tensor("s_out", [1, ROWS], f32, kind="ExternalOutput")

    LEAD = 3  # projection chunks emitted ahead of the scan

    with tile.TileContext(nc) as tc:
        with (
            tc.tile_pool(name="consts", bufs=1) as consts,
            tc.tile_pool(name="encp", bufs=3) as encp,
            tc.tile_pool(name="gpool", bufs=1) as gpool,
            tc.tile_pool(name="proj_ps", bufs=3, space="PSUM") as proj_ps,
            tc.tile_pool(name="scan_ps", bufs=2, space="PSUM") as scan_ps,
            tc.tile_pool(name="s_ps", bufs=1, space="PSUM") as s_ps,
        ):
            w_sb = consts.tile([128, 8, 128], bf16_t)
            expT_sb = consts.tile([128, 4, 128], bf16_t)
            bias_sb = consts.tile([128, 2], f32)
            expStart_sb = consts.tile([128, 2], f32)
            expEnd_sb = consts.tile([128, 2], bf16_t)
            s_sb = consts.tile([1, ROWS], f32)
            ring = consts.tile([128, RING, 2, BC], bf16_t)

            nc.sync.dma_start(out=w_sb[:], in_=wblk[:])
            nc.sync.dma_start(out=expT_sb[:], in_=expTblk[:])
            nc.sync.dma_start(out=bias_sb[:], in_=biasT[:])
            nc.sync.dma_start(out=expStart_sb[:], in_=expStartT[:])
            nc.sync.dma_start(out=expEnd_sb[:], in_=expEndT[:])

            # ---------------- projection (one chunk) ----------------
            gtiles = []

            def emit_proj_chunk(c):
                et = encp.tile([128, 4, CHUNK], bf16_t, name="et", tag="enc")
                nc.sync.dma_start(out=et[:], in_=encT[:, c, :, :])
                g = gpool.tile([128, 2, CHUNK], bf16_t, name=f"g{c}", tag=f"g{c}")
                gtiles.append(g)
                for vh in range(2):
                    ps = proj_ps.tile([128, CHUNK], f32, name="pps", tag="pps")
                    for ht in range(4):
                        nc.tensor.matmul(
                            ps[:],
                            lhsT=w_sb[:, ht * 2 + vh, :],
                            rhs=et[:, ht, :],
                            start=(ht == 0),
                            stop=(ht == 3),
                        )
                    nc.scalar.activation(
                        g[:, vh, :], ps[:],
                        mybir.ActivationFunctionType.Exp,
                        bias=bias_sb[:, vh:vh + 1], scale=1.0,
                    )

            def emit_sblock(k):
                # S_t for steps t in [k*SBLK, (k+1)*SBLK) from ring slots
                sp = s_ps.tile([1, SBLK * BC], f32, name="sps", tag="sps")
                s0 = (k * SBLK) % RING
                for ih in range(2):
                    nc.tensor.matmul(
                        sp[:],
                        lhsT=expEnd_sb[:, ih:ih + 1],
                        rhs=ring[:, s0:s0 + SBLK, ih, :],
                        start=(ih == 0),
                        stop=(ih == 1),
                    )
                nc.scalar.copy(
                    s_sb[0:1, k * (SBLK * BC):(k + 1) * (SBLK * BC)], sp[:])

            for c in range(LEAD):
                emit_proj_chunk(c)

            # ---------------- scan ----------------
            for ih in range(2):
                nc.vector.tensor_scalar_mul(
                    ring[:, 0, ih, :],
                    in0=gtiles[0][:, ih, 0:BC],
                    scalar1=expStart_sb[:, ih:ih + 1],
                )

            for t in range(1, S):
                gt = gtiles[t // SBLK]
                off = (t % SBLK) * BC
                # two psum banks (one per j-half) so the DVE multiply of one
                # half overlaps the matmuls of the other
                psA = scan_ps.tile([128, BC], f32, name="psA", tag="psA")
                psB = scan_ps.tile([128, BC], f32, name="psB", tag="psB")
                for jh, ps in ((0, psA), (1, psB)):
                    for ih in range(2):
                        nc.tensor.matmul(
                            ps[:],
                            lhsT=expT_sb[:, ih * 2 + jh, :],
                            rhs=ring[:, (t - 1) % RING, ih, :],
                            start=(ih == 0),
                            stop=(ih == 1),
                        )
                    nc.vector.tensor_tensor(
                        out=ring[:, t % RING, jh, :],
                        in0=ps[:],
                        in1=gt[:, jh, off:off + BC],
                        op=mybir.AluOpType.mult,
                    )
                if t % SBLK == SBLK - 1:
                    emit_sblock(t // SBLK)
                    if t // SBLK + LEAD < NCHUNK:
                        emit_proj_chunk(t // SBLK + LEAD)

            nc.sync.dma_start(out=s_out[:], in_=s_sb[:])

    nc.compile()
    return nc


def _host_consts(d):
    W_ = np.asarray(d["W"], dtype=np.float32)
    b_ = np.asarray(d["b"], dtype=np.float64)
    T_ = np.asarray(d["transition"], dtype=np.float64)
    start_ = np.asarray(d["start_transition"], dtype=np.float64)
    end_ = np.asarray(d["end_transition"], dtype=np.float64)
    Wb = np.ascontiguousarray(
        W_.reshape(4, 128, 2, 128).transpose(1, 0, 2, 3).reshape(128, 8, 128)
    ).astype(bf16)
    expTb = np.ascontiguousarray(
        np.exp(T_).reshape(2, 128, 2, 128).transpose(1, 0, 2, 3).reshape(128, 4, 128)
    ).astype(bf16)
    biasT = np.ascontiguousarray(
        (b_ - KAPPA).reshape(2, 128).T).astype(np.float32)
    expStartT = np.ascontiguousarray(
        np.exp(start_).reshape(2, 128).T).astype(np.float32)
    expEndT = np.ascontiguousarray(
        np.exp(end_).reshape(2, 128).T).astype(bf16)
    return Wb, expTb, biasT, expStartT, expEndT


def _prep_core_inputs(core, enc_bf, Wb, expTb, biasT, expStartT, expEndT):
    # encT layout [h%128, chunk, h//128, row-in-chunk]; rows are t*BC + b
    b0 = core * BC
    e = enc_bf[:, b0:b0 + BC, :].transpose(2, 0, 1).reshape(4, 128, NCHUNK, CHUNK)
    e = np.ascontiguousarray(e.transpose(1, 2, 0, 3))
    return {
        "encT": e, "wblk": Wb, "expTblk": expTb, "biasT": biasT,
        "expStartT": expStartT, "expEndT": expEndT,
    }


def kernel(enc_outs, W, b, transition, start_transition, end_transition,
           targets, lengths):
    global _nc_cache
    if _nc_cache is None:
        _nc_cache = _build()
    nc = _nc_cache

    enc = np.asarray(enc_outs, dtype=np.float32)
    W_ = np.asarray(W, dtype=np.float32)
    b_ = np.asarray(b, dtype=np.float64)
    T_ = np.asarray(transition, dtype=np.float64)
    start_ = np.asarray(start_transition, dtype=np.float64)
    end_ = np.asarray(end_transition, dtype=np.float64)
    tgt = np.asarray(targets).astype(np.int64)
    lens = np.asarray(lengths).astype(np.int64)

    Wb, expTb, biasT, expStartT, expEndT = _host_consts({
        "W": W, "b": b, "transition": transition,
        "start_transition": start_transition, "end_transition": end_transition,
    })
    enc_bf = enc.astype(bf16)
    in_maps = [
        _prep_core_inputs(c, enc_bf, Wb, expTb, biasT, expStartT, expEndT)
        for c in range(NCORES)
    ]
    res = run_bass_kernel_spmd(nc, in_maps, list(range(NCORES))).results

    # ---------------- host epilogue (small inputs only) ----------------
    tmask = (np.arange(S)[:, None] < lens[None, :])
    trans_sum = (T_[tgt[:-1], tgt[1:]] * tmask[1:]).sum(axis=0)
    last_tgt = tgt[lens - 1, np.arange(B)]
    hostscore = start_[tgt[0]] + trans_sum + end_[last_tgt]

    # gold-path raw emission scores: R[t, b, tgt] = enc[t, b] . W[:, tgt] + b
    # (16K dot products per core; 0.1% of the device FLOPs)
    Wg = W_.T[tgt.reshape(-1)]                        # (S*B, H)
    emis_all = (np.einsum("rh,rh->r", enc.reshape(S * B, H), Wg,
                          optimize=True).reshape(S, B)
                + b_[tgt])
    emis = ((emis_all - KAPPA) * tmask).sum(axis=0)

    loss_b = np.zeros(B, dtype=np.float64)
    for c in range(NCORES):
        b0 = c * BC
        s_flat = np.asarray(res[c]["s_out"], dtype=np.float64).reshape(ROWS)
        # S col layout: block k = t//SBLK, then g, then t%SBLK, then b%GB
        s_dec = s_flat.reshape(S // SBLK, NG, SBLK, GB)
        bl = lens[b0:b0 + BC] - 1
        blocal = np.arange(BC)
        s_end = s_dec[bl // SBLK, blocal // GB, bl % SBLK, blocal % GB]
        loss_b[b0:b0 + BC] = np.log(s_end) - emis[b0:b0 + BC] \
            - hostscore[b0:b0 + BC]

    return np.float32(loss_b.mean())


# revision 11
# speedup vs baseline: 1.1310x; 1.0082x over previous
"""CRF decoder loss kernel for Trainium2 (8 NeuronCores, data-parallel over batch).

Algorithm (mathematically identical to the reference):
  The reference computes mean_b(Zp - score) where Zp is the CRF partition
  function of log_softmax(enc@W+b) and score is the gold-path score. Writing
  logits = R - logZ (R the raw projection scores, logZ the log-softmax
  normalizer), the normalizer cancels between Zp and score, so no softmax is
  ever needed. With a constant shift kappa for range control, the forward
  recursion runs in LINEAR space:

      P_0 = exp(start) * G_0,     P_t = (P_{t-1} @ exp(T)) * G_t,
      G_t = exp(R_t - kappa)                                  (all [B, V])

  loss_b = log(sum_j P_{len_b-1}[b,j] * exp(end_j))           <- S, device
           - sum_{t<len_b} (R[t,b,tgt_{t,b}] - kappa)         <- host (tiny)
           - (start[tgt_0] + sum T[tgt,tgt'] + end[tgt_last]) <- host (tiny)

  Validated vs the reference: f64 exact (1e-16); with bf16 device dtypes the
  loss rel-err is ~1e-6.

Device work per core (batch shard of 32, v-major layouts):
  - projection: R^T = W^T @ encT into PSUM (bf16 matmuls, fp32 accum),
    ACT evicts G^T = exp(R^T + (b - kappa)) as bf16.
  - scan: state P^T [v, 32] bf16 in a 32-slot ring; per step 4 matmuls with
    the four 128x128 blocks of exp(T) stationary + one DVE multiply by G_t^T.
    Two independent 16-batch groups interleave to hide cross-engine latency.
  - S extraction: every 16 steps a batched matmul with exp(end) over the ring
    yields S_t[b] for all (t, b); host picks t = len_b - 1.
"""

import numpy as np
import ml_dtypes

import concourse.bacc as bacc
import concourse.tile as tile
from concourse import mybir
from concourse.bass_utils import run_bass_kernel_spmd

bf16 = ml_dtypes.bfloat16
f32 = mybir.dt.float32
bf16_t = mybir.dt.bfloat16

S, B, H, V = 512, 256, 512, 256
NCORES = 8
BC = B // NCORES            # 32 batch per core
ROWS = S * BC               # 16384 rows (t-major, b-minor)
KAPPA = 6.05
CHUNK = 512                 # projection chunk (rows)
NCHUNK = ROWS // CHUNK      # 32
NG = 2                      # scan batch groups per core
GB = BC // NG               # 16
SBLK = 16                   # scan steps per S-extraction block
RING = 32                   # state ring slots

_nc_cache = None


def _build():
    nc = bacc.Bacc("TRN2", debug=False)

    encT = nc.dram_tensor("encT", [128, NCHUNK, 4, CHUNK], bf16_t, kind="ExternalInput")
    wblk = nc.dram_tensor("wblk", [128, 8, 128], bf16_t, kind="ExternalInput")
    expTblk = nc.dram_tensor("expTblk", [128, 4, 128], mybir.dt.float8e4, kind="ExternalInput")
    biasT = nc.dram_tensor("biasT", [128, 2], f32, kind="ExternalInput")
    expStartT = nc.dram_tensor("expStartT", [128, 2], f32, kind="ExternalInput")
    expEndT = nc.dram_tensor("expEndT", [128, 2], bf16_t, kind="ExternalInput")

    s_out = nc.dram_tensor("s_out", [1, ROWS], f32, kind="ExternalOutput")

    LEAD = 3  # projection chunks emitted ahead of the scan

    with tile.TileContext(nc) as tc:
        with (
            tc.tile_pool(name="consts", bufs=1) as consts,
            tc.tile_pool(name="encp", bufs=3) as encp,
            tc.tile_pool(name="gpool", bufs=1) as gpool,
            tc.tile_pool(name="proj_ps", bufs=3, space="PSUM") as proj_ps,
            tc.tile_pool(name="scan_ps", bufs=2, space="PSUM") as scan_ps,
            tc.tile_pool(name="s_ps", bufs=1, space="PSUM") as s_ps,
        ):
            w_sb = consts.tile([128, 8, 128], bf16_t)
            expT_sb = consts.tile([128, 4, 128], mybir.dt.float8e4)
            bias_sb = consts.tile([128, 2], f32)
            expStart_sb = consts.tile([128, 2], f32)
            expEnd_sb = consts.tile([128, 2], bf16_t)
            s_sb = consts.tile([1, ROWS], f32)
            ring = consts.tile([128, RING, 2, BC], bf16_t)

            nc.sync.dma_start(out=w_sb[:], in_=wblk[:])
            nc.sync.dma_start(out=expT_sb[:], in_=expTblk[:])
            nc.sync.dma_start(out=bias_sb[:], in_=biasT[:])
            nc.sync.dma_start(out=expStart_sb[:], in_=expStartT[:])
            nc.sync.dma_start(out=expEnd_sb[:], in_=expEndT[:])

            # ---------------- projection (one chunk) ----------------
            gtiles = []

            def emit_proj_chunk(c):
                et = encp.tile([128, 4, CHUNK], bf16_t, name="et", tag="enc")
                nc.sync.dma_start(out=et[:], in_=encT[:, c, :, :])
                g = gpool.tile([128, 2, CHUNK], bf16_t, name=f"g{c}", tag=f"g{c}")
                gtiles.append(g)
                for vh in range(2):
                    ps = proj_ps.tile([128, CHUNK], f32, name="pps", tag="pps")
                    for ht in range(4):
                        nc.tensor.matmul(
                            ps[:],
                            lhsT=w_sb[:, ht * 2 + vh, :],
                            rhs=et[:, ht, :],
                            start=(ht == 0),
                            stop=(ht == 3),
                        )
                    nc.scalar.activation(
                        g[:, vh, :], ps[:],
                        mybir.ActivationFunctionType.Exp,
                        bias=bias_sb[:, vh:vh + 1], scale=1.0,
                    )

            def emit_sblock(k):
                # S_t for steps t in [k*SBLK, (k+1)*SBLK) from ring slots
                sp = s_ps.tile([1, SBLK * BC], f32, name="sps", tag="sps")
                s0 = (k * SBLK) % RING
                for ih in range(2):
                    nc.tensor.matmul(
                        sp[:],
                        lhsT=expEnd_sb[:, ih:ih + 1],
                        rhs=ring[:, s0:s0 + SBLK, ih, :],
                        start=(ih == 0),
                        stop=(ih == 1),
                    )
                nc.scalar.copy(
                    s_sb[0:1, k * (SBLK * BC):(k + 1) * (SBLK * BC)], sp[:])

            for c in range(LEAD):
                emit_proj_chunk(c)

            # ---------------- scan ----------------
            for ih in range(2):
                nc.vector.tensor_scalar_mul(
                    ring[:, 0, ih, :],
                    in0=gtiles[0][:, ih, 0:BC],
                    scalar1=expStart_sb[:, ih:ih + 1],
                )

            for t in range(1, S):
                gt = gtiles[t // SBLK]
                off = (t % SBLK) * BC
                # two psum banks (one per j-half) so the DVE multiply of one
                # half overlaps the matmuls of the other
                psA = scan_ps.tile([128, BC], f32, name="psA", tag="psA")
                psB = scan_ps.tile([128, BC], f32, name="psB", tag="psB")
                for jh, ps in ((0, psA), (1, psB)):
                    for ih in range(2):
                        nc.tensor.matmul(
                            ps[:],
                            lhsT=expT_sb[:, ih * 2 + jh, :],
                            rhs=ring[:, (t - 1) % RING, ih, :],
                            start=(ih == 0),
                            stop=(ih == 1),
                        )
                    nc.vector.tensor_tensor(
                        out=ring[:, t % RING, jh, :],
                        in0=ps[:],
                        in1=gt[:, jh, off:off + BC],
                        op=mybir.AluOpType.mult,
                    )
                if t % SBLK == SBLK - 1:
                    emit_sblock(t // SBLK)
                    if t // SBLK + LEAD < NCHUNK:
                        emit_proj_chunk(t // SBLK + LEAD)

            nc.sync.dma_start(out=s_out[:], in_=s_sb[:])

    nc.compile()
    return nc


def _host_consts(d):
    W_ = np.asarray(d["W"], dtype=np.float32)
    b_ = np.asarray(d["b"], dtype=np.float64)
    T_ = np.asarray(d["transition"], dtype=np.float64)
    start_ = np.asarray(d["start_transition"], dtype=np.float64)
    end_ = np.asarray(d["end_transition"], dtype=np.float64)
    Wb = np.ascontiguousarray(
        W_.reshape(4, 128, 2, 128).transpose(1, 0, 2, 3).reshape(128, 8, 128)
    ).astype(bf16)
    expTb = np.ascontiguousarray(
        np.exp(T_).reshape(2, 128, 2, 128).transpose(1, 0, 2, 3).reshape(128, 4, 128)
    ).astype(ml_dtypes.float8_e4m3fn)
    biasT = np.ascontiguousarray(
        (b_ - KAPPA).reshape(2, 128).T).astype(np.float32)
    expStartT = np.ascontiguousarray(
        np.exp(start_).reshape(2, 128).T).astype(np.float32)
    expEndT = np.ascontiguousarray(
        np.exp(end_).reshape(2, 128).T).astype(bf16)
    return Wb, expTb, biasT, expStartT, expEndT


def _prep_core_inputs(core, enc_bf, Wb, expTb, biasT, expStartT, expEndT):
    # encT layout [h%128, chunk, h//128, row-in-chunk]; rows are t*BC + b
    b0 = core * BC
    e = enc_bf[:, b0:b0 + BC, :].transpose(2, 0, 1).reshape(4, 128, NCHUNK, CHUNK)
    e = np.ascontiguousarray(e.transpose(1, 2, 0, 3))
    return {
        "encT": e, "wblk": Wb, "expTblk": expTb, "biasT": biasT,
        "expStartT": expStartT, "expEndT": expEndT,
    }


def kernel(enc_outs, W, b, transition, start_transition, end_transition,
           targets, lengths):
    global _nc_cache
    if _nc_cache is None:
        _nc_cache = _build()
    nc = _nc_cache

    enc = np.asarray(enc_outs, dtype=np.float32)
    W_ = np.asarray(W, dtype=np.float32)
    b_ = np.asarray(b, dtype=np.float64)
    T_ = np.asarray(transition, dtype=np.float64)
    start_ = np.asarray(start_transition, dtype=np.float64)
    end_ = np.asarray(end_transition, dtype=np.float64)
    tgt = np.asarray(targets).astype(np.int64)
    lens = np.asarray(lengths).astype(np.int64)

    Wb, expTb, biasT, expStartT, expEndT = _host_consts({
        "W": W, "b": b, "transition": transition,
        "start_transition": start_transition, "end_transition": end_transition,
    })
    enc_bf = enc.astype(bf16)
    in_maps = [
        _prep_core_inputs(c, enc_bf, Wb, expTb, biasT, expStartT, expEndT)
        for c in range(NCORES)
    ]
    res = run_bass_kernel_spmd(nc, in_maps, list(range(NCORES))).results

    # ---------------- host epilogue (small inputs only) ----------------
    tmask = (np.arange(S)[:, None] < lens[None, :])
    trans_sum = (T_[tgt[:-1], tgt[1:]] * tmask[1:]).sum(axis=0)
    last_tgt = tgt[lens - 1, np.arange(B)]
    hostscore = start_[tgt[0]] + trans_sum + end_[last_tgt]

    # gold-path raw emission scores: R[t, b, tgt] = enc[t, b] . W[:, tgt] + b
    # (16K dot products per core; 0.1% of the device FLOPs)
    Wg = W_.T[tgt.reshape(-1)]                        # (S*B, H)
    emis_all = (np.einsum("rh,rh->r", enc.reshape(S * B, H), Wg,
                          optimize=True).reshape(S, B)
                + b_[tgt])
    emis = ((emis_all - KAPPA) * tmask).sum(axis=0)

    loss_b = np.zeros(B, dtype=np.float64)
    for c in range(NCORES):
        b0 = c * BC
        s_flat = np.asarray(res[c]["s_out"], dtype=np.float64).reshape(ROWS)
        # S col layout: (t//SBLK) * 512 + (t%SBLK) * BC + b
        s_dec = s_flat.reshape(S // SBLK, SBLK, BC)
        bl = lens[b0:b0 + BC] - 1
        blocal = np.arange(BC)
        s_end = s_dec[bl // SBLK, bl % SBLK, blocal]
        loss_b[b0:b0 + BC] = np.log(s_end) - emis[b0:b0 + BC] \
            - hostscore[b0:b0 + BC]

    return np.float32(loss_b.mean())
